# revision 1
# baseline (speedup 1.0000x reference)
"""Binomial-deviance loss (cosine-similarity based) on 8 Trainium2 cores.

Strategy: data-parallel over the N=131072 pair dimension (16384 rows/core).
Per core, three per-row reductions over D=512 are computed from natural-layout
[128, F*512] tiles (F rows per partition):
  dot   = sum(o1*o2)  -> DVE tensor_mul + one segmented 3D tensor_reduce per
                         group (this walrus rejects tensor_tensor_reduce)
  n1    = sum(o1*o1)  -> first half: GPSIMD square + DVE segmented reduce;
                         second half: ACT Square with accum_out
  n2    = sum(o2*o2)  -> ACT Square with accum_out
Engine balance per group (measured): DVE ~11.3us, ACT ~11.1us, GPSIMD ~4.2us,
DMA ~13us -> memory-bound at the ~208us/core 8-core DMA floor.
Tail: d = dot*exp(-0.5*ln(n1*n2)) (no sqrt table needed), softplus terms via
ln(1+exp(x)) (no softplus table in this toolchain), masked sums -> [128,3]
partials per core; host reduces 8x128x3 floats and applies the final division.

This walrus build only accepts ONE semaphore wait per instruction, while Tile
emits multi-wait sync_info; a post-pass hoists overflow waits onto injected
same-engine InstNoOps.
"""

import sys

import numpy as np

if "/opt/trn_rl_repo" not in sys.path:
    try:
        import concourse  # noqa: F401
    except ImportError:
        sys.path.insert(0, "/opt/trn_rl_repo")

N, D = 131072, 512
NCORES = 8
CORE_ROWS = N // NCORES  # 16384
P = 128  # partitions
F = 8  # rows per partition per group
GROUP_ROWS = P * F  # 1024
G = CORE_ROWS // GROUP_ROWS  # 16
COLS = G * F  # 128 accumulator columns per partition
ALPHA = 50.0
BETA = 0.5

DATA_BUFS = 3

_CACHE = {}


def _split_waits(nc, mybir, maxw=1):
    """walrus here rejects >1 sync wait per instruction; hoist extras onto
    injected same-engine NoOps placed immediately before the instruction."""
    for fn in nc.m.functions:
        for blk in fn.blocks:
            new_insts = []
            for inst in blk.instructions:
                si = inst.sync_info
                if si is not None and si.on_wait and len(si.on_wait) > maxw:
                    waits = list(si.on_wait)
                    k = 0
                    while len(waits) - k > maxw:
                        chunk = waits[k : k + maxw]
                        k += maxw
                        nop = mybir.InstNoOp(
                            name=f"{inst.name}-ws{k}", ins=[], outs=[]
                        )
                        nop.engine = inst.engine
                        nop.sync_info = mybir.SyncInfo(on_wait=chunk, on_update=[])
                        new_insts.append(nop)
                    inst.sync_info = mybir.SyncInfo(
                        on_wait=waits[k:], on_update=list(si.on_update or [])
                    )
                new_insts.append(inst)
            blk.instructions = new_insts


def _build_nc():
    import concourse.bass as bass
    import concourse.mybir as mybir
    from concourse.tile import TileContext

    fp32 = mybir.dt.float32
    Act = mybir.ActivationFunctionType
    Alu = mybir.AluOpType

    nc = bass.Bass()
    o1 = nc.dram_tensor("o1", [CORE_ROWS, D], fp32, kind="ExternalInput")
    o2 = nc.dram_tensor("o2", [CORE_ROWS, D], fp32, kind="ExternalInput")
    mask = nc.dram_tensor("mask", [P, COLS], fp32, kind="ExternalInput")
    out = nc.dram_tensor("partials", [P, 3], fp32, kind="ExternalOutput")

    with TileContext(nc) as tc:
        with (
            tc.tile_pool(name="data", bufs=DATA_BUFS) as dpool,
            tc.tile_pool(name="acc", bufs=1) as apool,
            tc.tile_pool(name="scr", bufs=1) as spool,
            tc.tile_pool(name="gsq", bufs=2) as gpool,
            tc.tile_pool(name="psum", bufs=1, space="PSUM") as ppool,
        ):
            # Per group of F=8 segments (col = g*8+b), work split by engine:
            #   dot (all 8 segs)  -> DVE mult + one segmented reduce
            #   n1  b0..3 -> GPS square + DVE reduce;  n1 b4..7 -> ACT
            #   n2  (all) -> ACT Square+accum
            # Heavier GPS shares regress: GPS per-instr overhead is high and
            # its SBUF traffic contends with DMA writes (measured).
            HF = F // 2
            dot_acc = apool.tile([P, COLS], fp32, tag="dot_acc")
            n1d_acc = apool.tile([P, G * HF], fp32, tag="n1d_acc")
            n1a_acc = apool.tile([P, G * (F - HF)], fp32, tag="n1a_acc")
            n2_acc = apool.tile([P, COLS], fp32, tag="n2_acc")
            mask_t = apool.tile([P, COLS], fp32, tag="mask_t")
            negm_t = apool.tile([P, COLS], fp32, tag="negm_t")
            prod_scr = spool.tile([P, F * D], fp32, tag="prod_scr")
            act_scr = ppool.tile([P, D], fp32, tag="act_scr")

            nc.sync.dma_start(out=mask_t[:, :], in_=mask[:, :])
            nc.vector.tensor_scalar(
                out=negm_t[:, :],
                in0=mask_t[:, :],
                scalar1=-1.0,
                scalar2=1.0,
                op0=Alu.mult,
                op1=Alu.add,
            )

            o1v = o1[:, :].rearrange("(g p f) d -> g p (f d)", g=G, p=P, f=F)
            o2v = o2[:, :].rearrange("(g p f) d -> g p (f d)", g=G, p=P, f=F)

            def sq_accum(in0, acc_col):
                nc.scalar.activation(
                    out=act_scr[:, :],
                    in_=in0,
                    func=Act.Square,
                    accum_out=acc_col,
                )

            for g in range(G):
                t1 = dpool.tile([P, F * D], fp32, tag="t1")
                t2 = dpool.tile([P, F * D], fp32, tag="t2")
                gscr = gpool.tile([P, HF * D], fp32, tag="gscr")
                HW = HF * D  # half-group width
                if g in (0, G - 1):
                    # split the first group's loads+dot into halves so compute
                    # starts ~6us earlier during the startup ramp; same for the
                    # last group so the final serial chain begins on its first
                    # half while the second is still in flight
                    nc.sync.dma_start(out=t1[:, 0:HW], in_=o1v[g][:, 0:HW])
                    nc.sync.dma_start(out=t2[:, 0:HW], in_=o2v[g][:, 0:HW])
                    nc.sync.dma_start(out=t1[:, HW:], in_=o1v[g][:, HW:])
                    nc.sync.dma_start(out=t2[:, HW:], in_=o2v[g][:, HW:])
                    for h in range(2):
                        sl = slice(h * HW, (h + 1) * HW)
                        nc.vector.tensor_mul(
                            out=prod_scr[:, sl], in0=t1[:, sl], in1=t2[:, sl]
                        )
                        nc.vector.tensor_reduce(
                            out=dot_acc[:, g * F + h * HF : g * F + (h + 1) * HF],
                            in_=prod_scr[:, sl].rearrange("p (s f) -> p s f", s=HF),
                            axis=mybir.AxisListType.X,
                            op=Alu.add,
                        )
                else:
                    nc.sync.dma_start(out=t1[:, :], in_=o1v[g])
                    nc.sync.dma_start(out=t2[:, :], in_=o2v[g])
                    nc.vector.tensor_mul(out=prod_scr[:, :], in0=t1[:, :], in1=t2[:, :])
                    nc.vector.tensor_reduce(
                        out=dot_acc[:, g * F : (g + 1) * F],
                        in_=prod_scr[:, :].rearrange("p (s f) -> p s f", s=F),
                        axis=mybir.AxisListType.X,
                        op=Alu.add,
                    )
                # n1 first half: GPSIMD squares, DVE segmented reduce
                nc.gpsimd.tensor_mul(
                    out=gscr[:, :], in0=t1[:, 0 : HF * D], in1=t1[:, 0 : HF * D]
                )
                nc.vector.tensor_reduce(
                    out=n1d_acc[:, g * HF : (g + 1) * HF],
                    in_=gscr[:, :].rearrange("p (s f) -> p s f", s=HF),
                    axis=mybir.AxisListType.X,
                    op=Alu.add,
                )
                # ACT: n1 second half + all of n2
                for b in range(HF, F):
                    sq_accum(
                        t1[:, b * D : (b + 1) * D],
                        n1a_acc[:, g * (F - HF) + (b - HF) : g * (F - HF) + (b - HF) + 1],
                    )
                for b in range(F):
                    col = g * F + b
                    sq_accum(t2[:, b * D : (b + 1) * D], n2_acc[:, col : col + 1])

            # ---- tail ----
            b_pos = spool.tile([P, 1], fp32, tag="b_pos")
            b_neg = spool.tile([P, 1], fp32, tag="b_neg")
            nc.gpsimd.memset(b_pos[:, :], BETA / 2.0)
            nc.gpsimd.memset(b_neg[:, :], -2.0 * ALPHA)

            nn_t = apool.tile([P, COLS], fp32, tag="nn_t")
            ln_t = apool.tile([P, COLS], fp32, tag="ln_t")
            rs_t = apool.tile([P, COLS], fp32, tag="rs_t")
            d_t = apool.tile([P, COLS], fp32, tag="d_t")
            e_p = apool.tile([P, COLS], fp32, tag="e_p")
            e_n = apool.tile([P, COLS], fp32, tag="e_n")
            spp_t = apool.tile([P, COLS], fp32, tag="spp_t")
            spn_t = apool.tile([P, COLS], fp32, tag="spn_t")
            f_scr = spool.tile([P, COLS], fp32, tag="f_scr")
            out_t = apool.tile([P, 3], fp32, tag="out_t")

            one = nc.const_aps.scalar_like(1.0, nn_t[:, :])

            # nn = n1*n2 with n1 split: cols 8g+[0,HF) in n1d_acc (DVE),
            # cols 8g+[HF,F) in n1a_acc (ACT)
            nn_v = nn_t[:, :].rearrange("p (g m) -> p g m", m=F)
            n2_v = n2_acc[:, :].rearrange("p (g m) -> p g m", m=F)
            n1d_v = n1d_acc[:, :].rearrange("p (g m) -> p g m", m=HF)
            n1a_v = n1a_acc[:, :].rearrange("p (g m) -> p g m", m=F - HF)
            nc.vector.tensor_mul(
                out=nn_v[:, :, 0:HF], in0=n1d_v, in1=n2_v[:, :, 0:HF]
            )
            nc.vector.tensor_mul(
                out=nn_v[:, :, HF:F], in0=n1a_v, in1=n2_v[:, :, HF:F]
            )
            # 1/sqrt(nn) = exp(-0.5*ln(nn)); no sqrt table switch needed --
            # ln/exp/square live in one ACT table set.
            nc.scalar.activation(out=ln_t[:, :], in_=nn_t[:, :], func=Act.Ln)
            nc.scalar.activation(
                out=rs_t[:, :], in_=ln_t[:, :], func=Act.Exp, scale=-0.5
            )
            nc.vector.tensor_mul(out=d_t[:, :], in0=dot_acc[:, :], in1=rs_t[:, :])
            # pos = (2/B)*softplus(-B*d + B/2); neg = (2/A)*softplus(A*d - 2A)
            # softplus(x) = ln(1 + exp(x))
            nc.scalar.activation(
                out=e_p[:, :], in_=d_t[:, :], func=Act.Exp,
                bias=b_pos[:, :], scale=-BETA,
            )
            nc.scalar.activation(out=spp_t[:, :], in_=e_p[:, :], func=Act.Ln, bias=one)
            nc.scalar.activation(
                out=e_n[:, :], in_=d_t[:, :], func=Act.Exp,
                bias=b_neg[:, :], scale=ALPHA,
            )
            nc.scalar.activation(out=spn_t[:, :], in_=e_n[:, :], func=Act.Ln, bias=one)
            # masked sums: multiply by mask then reduce (scale folded in via
            # tensor_scalar on the product)
            nc.vector.tensor_mul(out=f_scr[:, :], in0=spp_t[:, :], in1=mask_t[:, :])
            nc.vector.tensor_reduce(
                out=out_t[:, 0:1], in_=f_scr[:, :],
                axis=mybir.AxisListType.X, op=Alu.add,
            )
            nc.vector.tensor_mul(out=f_scr[:, :], in0=spn_t[:, :], in1=negm_t[:, :])
            nc.vector.tensor_reduce(
                out=out_t[:, 1:2], in_=f_scr[:, :],
                axis=mybir.AxisListType.X, op=Alu.add,
            )
            nc.vector.tensor_reduce(
                out=out_t[:, 2:3], in_=mask_t[:, :],
                axis=mybir.AxisListType.X, op=Alu.add,
            )
            nc.sync.dma_start(out=out[:, :], in_=out_t[:, :])

    _split_waits(nc, mybir, maxw=1)
    return nc


def _get_nc():
    if "nc" not in _CACHE:
        _CACHE["nc"] = _build_nc()
    return _CACHE["nc"]


def _make_in_maps(output1, output2, target):
    o1 = np.ascontiguousarray(output1, dtype=np.float32)
    o2 = np.ascontiguousarray(output2, dtype=np.float32)
    mask_full = (np.asarray(target) == 1).astype(np.float32)
    in_maps = []
    for c in range(NCORES):
        sl = slice(c * CORE_ROWS, (c + 1) * CORE_ROWS)
        m = mask_full[sl].reshape(G, P, F).transpose(1, 0, 2).reshape(P, COLS)
        in_maps.append(
            {"o1": o1[sl], "o2": o2[sl], "mask": np.ascontiguousarray(m)}
        )
    return in_maps


def _combine(results):
    parts = np.stack([r["partials"] for r in results]).astype(np.float64)
    pos_sum, neg_sum, num_pos = parts.sum(axis=(0, 1))
    num_pos = int(round(num_pos))
    num_neg = N - num_pos
    pos_loss = np.float32((2.0 / BETA) * pos_sum) / np.float32(max(num_pos, 1))
    neg_loss = np.float32((2.0 / ALPHA) * neg_sum) / np.float32(max(num_neg, 1))
    return np.float32(pos_loss + neg_loss)


def _run(output1, output2, target, trace=False, **spmd_kwargs):
    from concourse.bass_utils import run_bass_kernel_spmd

    nc = _get_nc()
    in_maps = _make_in_maps(output1, output2, target)
    res = run_bass_kernel_spmd(
        nc, in_maps, core_ids=list(range(NCORES)), trace=trace, **spmd_kwargs
    )
    return _combine(res.results), res


def kernel(output1, output2, target):
    try:
        loss, _ = _run(output1, output2, target, trace=False)
    except Exception:
        # transient NRT/device hiccups (e.g. NRT_EXEC_UNIT_UNRECOVERABLE)
        # usually clear on retry
        import time

        time.sleep(2.0)
        loss, _ = _run(output1, output2, target, trace=False)
    return loss



# revision 10
# speedup vs baseline: 1.5714x; 1.5714x over previous
"""Binomial-deviance loss (cosine-similarity based) on 8 Trainium2 cores.

v2: bf16 inputs + transposed layout + PE-matmul reductions.

The 2e-2 rel-err budget is ~4 orders of magnitude above what fp32 gives, so
inputs are downcast to bf16 on the host (halves HBM traffic: 67MB -> 33.5MB
per core, DMA floor ~104us at the measured 322GB/s/core). The host also
pre-transposes each core slice to d-major [512, 16384] so the per-row
reductions over D=512 become partition-axis reductions, which the Tensor
engine does via ones-vector matmuls (moving throughput 512 cols / 512 cyc
@2.4GHz) -- freeing the DVE from its 1x-only tensor_reduce.

Per core (16 row-tiles of 1024 rows, 4 d-chunks of 128 partitions each):
  DVE: prod = o1*o2 (bf16 TT 2x, one [128,4096] instr/rt) + sq2 = o2*o2
  ACT: sq1 = o1*o1 (Square, dtype-independent 1x)
  PE : dot/n1/n2 = ones[128,32]^T @ {prod,sq1,sq2} -> [32,512] PSUM stripes
       (32 replicated rows; M=32 because matmul output base partition must
       be 0/32/64 and engine APs reject partition strides, so replication
       makes the drain a contiguous [0:96] read): stripe (target t,
       row-block q) at bank q, partitions [32t, 32t+32), accumulated over
       the 4 d-chunks
  ACT: drain per 2-rt half-round: copy psum[0:96, h*2048:(h+1)*2048] ->
       SBUF stage [96,2048] (PSUM is not DMA-able in this stack); banks
       ping-pong in halves of 4
  DMA: scatter stage -> acc[128, 3*128] in natural row order (row r of the
       core lands at partition r//128, col r%128)
Tail on [128,128] tiles: d = dot*exp(-0.5*ln(n1*n2)), softplus terms via
ln(1+exp(x)), masked sums -> [128,3] partials; host reduces 8x128x3 and
applies the final divisions.

This walrus build only accepts ONE semaphore wait per instruction, while Tile
emits multi-wait sync_info; a post-pass hoists overflow waits onto injected
same-engine InstNoOps.
"""

import sys

import numpy as np

if "/opt/trn_rl_repo" not in sys.path:
    try:
        import concourse  # noqa: F401
    except ImportError:
        sys.path.insert(0, "/opt/trn_rl_repo")

N, D = 131072, 512
NCORES = 8
CORE_ROWS = N // NCORES  # 16384
P = 128  # partitions
NCHUNK = D // P  # 4 d-chunks
RT = 1024  # rows per tile
NRT = CORE_ROWS // RT  # 16 row tiles
ROUND_RT = 2  # row tiles per psum drain round
NROUND = NRT // ROUND_RT  # 8
ALPHA = 50.0
BETA = 0.5

_CACHE = {}


def _split_waits(nc, mybir, maxw=1):
    """walrus here rejects >1 sync wait per instruction; hoist extras onto
    injected same-engine NoOps placed immediately before the instruction."""
    for fn in nc.m.functions:
        for blk in fn.blocks:
            new_insts = []
            for inst in blk.instructions:
                si = inst.sync_info
                if si is not None and si.on_wait and len(si.on_wait) > maxw:
                    waits = list(si.on_wait)
                    k = 0
                    while len(waits) - k > maxw:
                        chunk = waits[k : k + maxw]
                        k += maxw
                        nop = mybir.InstNoOp(
                            name=f"{inst.name}-ws{k}", ins=[], outs=[]
                        )
                        nop.engine = inst.engine
                        nop.sync_info = mybir.SyncInfo(on_wait=chunk, on_update=[])
                        new_insts.append(nop)
                    inst.sync_info = mybir.SyncInfo(
                        on_wait=waits[k:], on_update=list(si.on_update or [])
                    )
                new_insts.append(inst)
            blk.instructions = new_insts


def _build_nc():
    import concourse.bass as bass
    import concourse.mybir as mybir
    from concourse.tile import TileContext

    fp32 = mybir.dt.float32
    bf16 = mybir.dt.bfloat16
    Act = mybir.ActivationFunctionType
    Alu = mybir.AluOpType

    TW = NCHUNK * RT  # 4096: tile width (4 chunks x 1024 rows)

    nc = bass.Bass()
    o1 = nc.dram_tensor("o1", [D, CORE_ROWS], bf16, kind="ExternalInput")
    o2 = nc.dram_tensor("o2", [D, CORE_ROWS], bf16, kind="ExternalInput")
    mask = nc.dram_tensor("mask", [P, P], fp32, kind="ExternalInput")
    out = nc.dram_tensor("partials", [P, 3], fp32, kind="ExternalOutput")

    with TileContext(nc) as tc:
        with (
            tc.tile_pool(name="data", bufs=2) as dpool,
            tc.tile_pool(name="work", bufs=2) as wpool,
            tc.tile_pool(name="stg", bufs=2) as spool,
            tc.tile_pool(name="acc", bufs=1) as apool,
            tc.tile_pool(name="psum", bufs=1, space="PSUM") as ppool,
        ):
            mask_t = apool.tile([P, P], fp32, tag="mask_t")
            negm_t = apool.tile([P, P], fp32, tag="negm_t")
            ones_t = apool.tile([P, 32], bf16, tag="ones_t")
            acc_t = apool.tile([P, 3 * P], fp32, tag="acc_t")

            nc.sync.dma_start(out=mask_t[:, :], in_=mask[:, :])
            nc.vector.tensor_scalar(
                out=negm_t[:, :],
                in0=mask_t[:, :],
                scalar1=-1.0,
                scalar2=1.0,
                op0=Alu.mult,
                op1=Alu.add,
            )
            nc.gpsimd.memset(ones_t[:, :], 1.0)

            # dram views: [rt, partition(d%128), chunk(d//128), row-in-tile]
            o1v = o1[:, :].rearrange(
                "(c p) (t r) -> t p c r", c=NCHUNK, p=P, t=NRT, r=RT
            )
            o2v = o2[:, :].rearrange(
                "(c p) (t r) -> t p c r", c=NCHUNK, p=P, t=NRT, r=RT
            )

            # all 8 PSUM banks: bank q = row-block within a 4-rt round,
            # partition offset 32*t = target (dot/n1/n2)
            ps_t = ppool.tile([P, 8 * 512], fp32, tag="ps")
            for rt in range(NRT):
                t1 = dpool.tile([P, TW], bf16, tag="t1")
                t2 = dpool.tile([P, TW], bf16, tag="t2")
                prod = wpool.tile([P, TW], bf16, tag="prod")
                sq1 = wpool.tile([P, TW], bf16, tag="sq1")
                sq2 = wpool.tile([P, TW], bf16, tag="sq2")
                t1v = t1[:, :].rearrange("p (c r) -> p c r", c=NCHUNK)
                t2v = t2[:, :].rearrange("p (c r) -> p c r", c=NCHUNK)
                nc.sync.dma_start(out=t1v, in_=o1v[rt])
                nc.sync.dma_start(out=t2v, in_=o2v[rt])

                nc.vector.tensor_mul(out=prod[:, :], in0=t1[:, :], in1=t2[:, :])
                nc.scalar.activation(out=sq1[:, :], in_=t1[:, :], func=Act.Square)
                nc.vector.tensor_mul(out=sq2[:, :], in0=t2[:, :], in1=t2[:, :])

                for ti, src in enumerate((prod, sq1, sq2)):
                    for j in range(RT // 512):
                        q = (rt % 4) * 2 + j  # bank within the 4-rt round
                        for c in range(NCHUNK):
                            nc.tensor.matmul(
                                out=ps_t[32 * ti : 32 * ti + 32, q * 512 : (q + 1) * 512],
                                lhsT=ones_t[:, :],
                                rhs=src[:, c * RT + j * 512 : c * RT + j * 512 + 512],
                                start=(c == 0),
                                stop=(c == NCHUNK - 1),
                            )

                if rt % 2 == 1:
                    hr = rt // 2  # half-round: banks [h*4, h*4+4) of this round
                    h = hr % 2
                    stage = spool.tile([96, 2048], fp32, tag="stage")
                    nc.scalar.copy(stage[:, :], ps_t[0:96, h * 2048 : (h + 1) * 2048])
                    # scatter to natural row order: row r -> acc[r//128, r%128]
                    # (dma matches flat element order, so a contiguous [1,2048]
                    # src feeds a [16,128] dst directly)
                    for ti in range(3):
                        nc.sync.dma_start(
                            out=acc_t[hr * 16 : (hr + 1) * 16, ti * P : (ti + 1) * P],
                            in_=stage[32 * ti : 32 * ti + 1, :],
                        )

            # ---- tail ----
            b_pos = apool.tile([P, 1], fp32, tag="b_pos")
            b_neg = apool.tile([P, 1], fp32, tag="b_neg")
            nc.gpsimd.memset(b_pos[:, :], BETA / 2.0)
            nc.gpsimd.memset(b_neg[:, :], -2.0 * ALPHA)

            dot_a = acc_t[:, 0:P]
            n1_a = acc_t[:, P : 2 * P]
            n2_a = acc_t[:, 2 * P : 3 * P]

            nn_t = apool.tile([P, P], fp32, tag="nn_t")
            rs_t = apool.tile([P, P], fp32, tag="rs_t")
            d_t = apool.tile([P, P], fp32, tag="d_t")
            e_t = apool.tile([P, P], fp32, tag="e_t")
            sp_t = apool.tile([P, P], fp32, tag="sp_t")
            f_t = apool.tile([P, P], fp32, tag="f_t")
            out_t = apool.tile([P, 3], fp32, tag="out_t")
            one = nc.const_aps.scalar_like(1.0, nn_t[:, :])

            nc.vector.tensor_mul(out=nn_t[:, :], in0=n1_a, in1=n2_a)
            # 1/sqrt(nn) = exp(-0.5*ln(nn)); ln/exp live in one ACT table set
            nc.scalar.activation(out=rs_t[:, :], in_=nn_t[:, :], func=Act.Ln)
            nc.scalar.activation(
                out=rs_t[:, :], in_=rs_t[:, :], func=Act.Exp, scale=-0.5
            )
            nc.vector.tensor_mul(out=d_t[:, :], in0=dot_a, in1=rs_t[:, :])
            # pos = (2/B)*softplus(-B*d + B/2); neg = (2/A)*softplus(A*d - 2A)
            nc.scalar.activation(
                out=e_t[:, :], in_=d_t[:, :], func=Act.Exp,
                bias=b_pos[:, :], scale=-BETA,
            )
            nc.scalar.activation(out=sp_t[:, :], in_=e_t[:, :], func=Act.Ln, bias=one)
            nc.vector.tensor_mul(out=f_t[:, :], in0=sp_t[:, :], in1=mask_t[:, :])
            nc.vector.tensor_reduce(
                out=out_t[:, 0:1], in_=f_t[:, :],
                axis=mybir.AxisListType.X, op=Alu.add,
            )
            nc.scalar.activation(
                out=e_t[:, :], in_=d_t[:, :], func=Act.Exp,
                bias=b_neg[:, :], scale=ALPHA,
            )
            nc.scalar.activation(out=sp_t[:, :], in_=e_t[:, :], func=Act.Ln, bias=one)
            nc.vector.tensor_mul(out=f_t[:, :], in0=sp_t[:, :], in1=negm_t[:, :])
            nc.vector.tensor_reduce(
                out=out_t[:, 1:2], in_=f_t[:, :],
                axis=mybir.AxisListType.X, op=Alu.add,
            )
            nc.vector.tensor_reduce(
                out=out_t[:, 2:3], in_=mask_t[:, :],
                axis=mybir.AxisListType.X, op=Alu.add,
            )
            nc.sync.dma_start(out=out[:, :], in_=out_t[:, :])

    _split_waits(nc, mybir, maxw=1)
    return nc


def _get_nc():
    if "nc" not in _CACHE:
        _CACHE["nc"] = _build_nc()
    return _CACHE["nc"]


def _make_in_maps(output1, output2, target):
    import ml_dtypes

    bf = ml_dtypes.bfloat16
    o1 = np.asarray(output1, dtype=np.float32).astype(bf)
    o2 = np.asarray(output2, dtype=np.float32).astype(bf)
    mask_full = (np.asarray(target) == 1).astype(np.float32)
    in_maps = []
    for c in range(NCORES):
        sl = slice(c * CORE_ROWS, (c + 1) * CORE_ROWS)
        in_maps.append(
            {
                "o1": np.ascontiguousarray(o1[sl].T),
                "o2": np.ascontiguousarray(o2[sl].T),
                "mask": mask_full[sl].reshape(P, P),
            }
        )
    return in_maps


def _combine(results):
    parts = np.stack([r["partials"] for r in results]).astype(np.float64)
    pos_sum, neg_sum, num_pos = parts.sum(axis=(0, 1))
    num_pos = int(round(num_pos))
    num_neg = N - num_pos
    pos_loss = np.float32((2.0 / BETA) * pos_sum) / np.float32(max(num_pos, 1))
    neg_loss = np.float32((2.0 / ALPHA) * neg_sum) / np.float32(max(num_neg, 1))
    return np.float32(pos_loss + neg_loss)


def _run(output1, output2, target, trace=False, **spmd_kwargs):
    from concourse.bass_utils import run_bass_kernel_spmd

    nc = _get_nc()
    in_maps = _make_in_maps(output1, output2, target)
    res = run_bass_kernel_spmd(
        nc, in_maps, core_ids=list(range(NCORES)), trace=trace, **spmd_kwargs
    )
    return _combine(res.results), res


def kernel(output1, output2, target):
    try:
        loss, _ = _run(output1, output2, target, trace=False)
    except Exception:
        # transient NRT/device hiccups (e.g. NRT_EXEC_UNIT_UNRECOVERABLE)
        # usually clear on retry
        import time

        time.sleep(2.0)
        loss, _ = _run(output1, output2, target, trace=False)
    return loss
